# revision 1
# baseline (speedup 1.0000x reference)
import numpy as np

import concourse.bass as bass
import concourse.mybir as mybir
import concourse.tile as tile
from concourse.bass_utils import run_bass_kernel_spmd

F32 = mybir.dt.float32
F32R = mybir.dt.float32r
AX = mybir.AxisListType
AF = mybir.ActivationFunctionType
OP = mybir.AluOpType

H, DH, C, T = 16, 64, 1024, 2048
NCORES = 8
EPS = 1e-5
S = 128          # tokens per strip per batch (A kernel)
MS = 512         # rows per strip (B kernel)


def _bc_last(ap, n):
    return bass.AP(tensor=ap.tensor, offset=ap.offset, ap=[*ap.ap, [0, n]])


def _bc_mid(ap, n):
    return bass.AP(
        tensor=ap.tensor, offset=ap.offset, ap=[ap.ap[0], [0, n], *ap.ap[1:]]
    )


def build_prog_a():
    nc = bass.Bass(use_seq_codegen=True)
    qT = nc.dram_tensor("qT", [8, 128, 2 * S], F32R, kind="ExternalInput")
    kT = nc.dram_tensor("kT", [8, 128, S], F32R, kind="ExternalInput")
    vT = nc.dram_tensor("vT", [8, 128, S], F32R, kind="ExternalInput")
    WqT = nc.dram_tensor("WqT", [8, 128, C], F32R, kind="ExternalInput")
    WkT = nc.dram_tensor("WkT", [8, 128, C], F32R, kind="ExternalInput")
    WvT = nc.dram_tensor("WvT", [8, 128, C], F32R, kind="ExternalInput")
    gq = nc.dram_tensor("gq", [128, C], F32, kind="ExternalInput")
    bq = nc.dram_tensor("bq", [128, C], F32, kind="ExternalInput")
    gk = nc.dram_tensor("gk", [128, C], F32, kind="ExternalInput")
    bk = nc.dram_tensor("bk", [128, C], F32, kind="ExternalInput")
    xout = nc.dram_tensor("xout", [2 * S, C], F32, kind="ExternalOutput")

    with tile.TileContext(nc) as tc:
        with (
            tc.tile_pool(name="wp", bufs=1) as wp,
            tc.tile_pool(name="mid", bufs=2) as mid,
            tc.tile_pool(name="st", bufs=4) as st,
            tc.tile_pool(name="pp", bufs=4, space="PSUM") as pp,
        ):
            wsb = {}
            for nm, drt in (("q", WqT), ("k", WkT), ("v", WvT)):
                w = wp.tile([128, 8, C], F32R, tag="w" + nm)
                for kb in range(8):
                    nc.gpsimd.dma_start(out=w[:, kb, :], in_=drt[kb])
                wsb[nm] = w
            acts = {}
            for nm, drt, ntok in (("q", qT, 2 * S), ("k", kT, S), ("v", vT, S)):
                a = wp.tile([128, 8, ntok], F32R, tag="a" + nm)
                for kb in range(8):
                    nc.gpsimd.dma_start(out=a[:, kb, :], in_=drt[kb])
                acts[nm] = a
            lnp = {}
            for nm, drt in (("gq", gq), ("bq", bq), ("gk", gk), ("bk", bk)):
                t_ = wp.tile([128, C], F32, tag=nm)
                nc.gpsimd.dma_start(out=t_, in_=drt[:, :])
                lnp[nm] = t_
            epst = wp.tile([128, 1], F32, tag="eps")
            nc.vector.memset(epst, EPS)

            def project(act, w, tok0, g_sb, b_sb, tag):
                ps = pp.tile([128, C], F32, tag="pj")
                for dh in range(2):
                    for kb in range(8):
                        nc.tensor.matmul(
                            ps[:, dh * 512 : (dh + 1) * 512],
                            lhsT=act[:, kb, tok0 : tok0 + 128],
                            rhs=w[:, kb, dh * 512 : (dh + 1) * 512],
                            start=(kb == 0),
                            stop=(kb == 7),
                        )
                sb = mid.tile([128, C], F32, tag=tag)
                if g_sb is None:
                    nc.scalar.copy(out=sb, in_=ps)
                    return sb
                ps3 = ps.rearrange("p (h d) -> p h d", d=DH)
                mu = st.tile([128, H], F32, tag="mu")
                nc.vector.reduce_sum(out=mu, in_=ps3, axis=AX.X)
                nc.vector.tensor_scalar_mul(mu, mu, 1.0 / DH)
                sb3 = sb.rearrange("p (h d) -> p h d", d=DH)
                nc.vector.tensor_tensor(
                    out=sb3, in0=ps3, in1=_bc_last(mu, DH), op=OP.subtract
                )
                sq = mid.tile([128, C], F32, tag="sq")
                sq3 = sq.rearrange("p (h d) -> p h d", d=DH)
                nc.vector.tensor_tensor(out=sq3, in0=sb3, in1=sb3, op=OP.mult)
                var = st.tile([128, H], F32, tag="var")
                nc.vector.reduce_sum(out=var, in_=sq3, axis=AX.X)
                nc.scalar.activation(
                    out=var, in_=var, func=AF.Sqrt, bias=epst, scale=1.0 / DH
                )
                nc.vector.reciprocal(var, var)
                nc.vector.tensor_tensor(
                    out=sb3, in0=sb3, in1=_bc_last(var, DH), op=OP.mult
                )
                nc.vector.tensor_tensor(out=sb, in0=sb, in1=g_sb, op=OP.mult)
                nc.vector.tensor_tensor(out=sb, in0=sb, in1=b_sb, op=OP.add)
                return sb

            for kt in range(S // 128):
                kp = project(acts["k"], wsb["k"], kt * 128, lnp["gk"], lnp["bk"], "kp")
                vp = project(acts["v"], wsb["v"], kt * 128, None, None, "vp")
                for b in range(2):
                    tok0 = b * S + kt * 128
                    qp = project(acts["q"], wsb["q"], tok0, lnp["gq"], lnp["bq"], "qp")
                    qp3 = qp.rearrange("p (h d) -> p h d", d=DH)
                    s = st.tile([128, H, H], F32, tag="s")
                    prod = mid.tile([128, C], F32, tag="prod")
                    prod3 = prod.rearrange("p (h d) -> p h d", d=DH)
                    for g in range(H):
                        kpg = _bc_mid(kp[:, g * DH : (g + 1) * DH], H)
                        nc.vector.tensor_tensor(out=prod3, in0=qp3, in1=kpg, op=OP.mult)
                        nc.vector.reduce_sum(out=s[:, :, g], in_=prod3, axis=AX.X)
                    mx = st.tile([128, H], F32, tag="mx")
                    nc.vector.reduce_max(out=mx, in_=s, axis=AX.X)
                    nc.vector.tensor_tensor(
                        out=s, in0=s, in1=_bc_last(mx, H), op=OP.subtract
                    )
                    nc.scalar.activation(out=s, in_=s, func=AF.Exp)
                    zz = st.tile([128, H], F32, tag="zz")
                    nc.vector.reduce_sum(out=zz, in_=s, axis=AX.X)
                    nc.vector.reciprocal(zz, zz)
                    nc.vector.tensor_tensor(
                        out=s, in0=s, in1=_bc_last(zz, H), op=OP.mult
                    )
                    x = mid.tile([128, C], F32, tag="x")
                    x3 = x.rearrange("p (h d) -> p h d", d=DH)
                    for g in range(H):
                        vpg = _bc_mid(vp[:, g * DH : (g + 1) * DH], H)
                        ag = _bc_last(s[:, :, g], DH)
                        if g == 0:
                            nc.vector.tensor_tensor(out=x3, in0=vpg, in1=ag, op=OP.mult)
                        else:
                            nc.vector.tensor_tensor(
                                out=prod3, in0=vpg, in1=ag, op=OP.mult
                            )
                            nc.vector.tensor_tensor(out=x3, in0=x3, in1=prod3, op=OP.add)
                    nc.gpsimd.dma_start(out=xout[tok0 : tok0 + 128, :], in_=x)
    return nc


def build_prog_b():
    nc = bass.Bass(use_seq_codegen=True)
    xrT = nc.dram_tensor("xrT", [8, 128, MS], F32R, kind="ExternalInput")
    WoT = nc.dram_tensor("WoT", [8, 128, C], F32R, kind="ExternalInput")
    bo = nc.dram_tensor("bo", [128, C], F32, kind="ExternalInput")
    o = nc.dram_tensor("o", [MS, C], F32, kind="ExternalOutput")
    with tile.TileContext(nc) as tc:
        with (
            tc.tile_pool(name="wp", bufs=1) as wp,
            tc.tile_pool(name="mid", bufs=4) as mid,
            tc.tile_pool(name="pp", bufs=4, space="PSUM") as pp,
        ):
            w = wp.tile([128, 8, C], F32R, tag="w")
            for kb in range(8):
                nc.gpsimd.dma_start(out=w[:, kb, :], in_=WoT[kb])
            a = wp.tile([128, 8, MS], F32R, tag="a")
            for kb in range(8):
                nc.gpsimd.dma_start(out=a[:, kb, :], in_=xrT[kb])
            bos = wp.tile([128, C], F32, tag="bo")
            nc.gpsimd.dma_start(out=bos, in_=bo[:, :])
            for mt in range(MS // 128):
                ps = pp.tile([128, C], F32, tag="pj")
                for dh in range(2):
                    for kb in range(8):
                        nc.tensor.matmul(
                            ps[:, dh * 512 : (dh + 1) * 512],
                            lhsT=a[:, kb, mt * 128 : (mt + 1) * 128],
                            rhs=w[:, kb, dh * 512 : (dh + 1) * 512],
                            start=(kb == 0),
                            stop=(kb == 7),
                        )
                osb = mid.tile([128, C], F32, tag="osb")
                nc.vector.scalar_tensor_tensor(
                    out=osb, in0=ps, scalar=1.0, in1=bos, op0=OP.mult, op1=OP.add
                )
                nc.gpsimd.dma_start(out=o[mt * 128 : (mt + 1) * 128, :], in_=osb)
    return nc


_PROGS = {}


def _get_progs():
    if "a" not in _PROGS:
        _PROGS["a"] = build_prog_a()
        _PROGS["b"] = build_prog_b()
    return _PROGS["a"], _PROGS["b"]


def _kernel_device(q, k, v, Wq, Wk, Wv, Wo, bo, gamma, beta):
    q = np.asarray(q, np.float32)
    k = np.asarray(k, np.float32)
    v = np.asarray(v, np.float32)
    Wo = np.asarray(Wo, np.float32)
    bo = np.asarray(bo, np.float32)
    gamma = np.asarray(gamma, np.float32)
    beta = np.asarray(beta, np.float32)
    scale = 8.0 / DH

    nc_a, nc_b = _get_progs()

    WqT = np.ascontiguousarray(np.asarray(Wq, np.float32).T).reshape(8, 128, C)
    WkT = np.ascontiguousarray(np.asarray(Wk, np.float32).T).reshape(8, 128, C)
    WvT = np.ascontiguousarray(np.asarray(Wv, np.float32).T).reshape(8, 128, C)
    WoT = np.ascontiguousarray(Wo.T).reshape(8, 128, C)
    gq = np.ascontiguousarray(np.broadcast_to(np.tile(gamma, H) * scale, (128, C)))
    bq = np.ascontiguousarray(np.broadcast_to(np.tile(beta, H) * scale, (128, C)))
    gk = np.ascontiguousarray(np.broadcast_to(np.tile(gamma, H), (128, C)))
    bk = np.ascontiguousarray(np.broadcast_to(np.tile(beta, H), (128, C)))
    bof = np.ascontiguousarray(np.broadcast_to(bo, (128, C)))

    # transposed activations per core: [C, 2, T] / [C, T]
    qTc = []
    for c in range(NCORES):
        qc = np.stack([q[c], q[c + 8]], axis=0)  # [2, T, C]
        qTc.append(np.ascontiguousarray(qc.transpose(2, 0, 1)))
    kTc = [np.ascontiguousarray(k[c % 4].T) for c in range(NCORES)]
    vTc = [np.ascontiguousarray(v[c % 4].T) for c in range(NCORES)]

    x_full = np.empty((NCORES, 2, T, C), np.float32)
    for sidx in range(T // S):
        t0 = sidx * S
        in_maps = []
        for c in range(NCORES):
            qTs = np.ascontiguousarray(
                qTc[c][:, :, t0 : t0 + S].reshape(C, 2 * S)
            ).reshape(8, 128, 2 * S)
            kTs = np.ascontiguousarray(kTc[c][:, t0 : t0 + S]).reshape(8, 128, S)
            vTs = np.ascontiguousarray(vTc[c][:, t0 : t0 + S]).reshape(8, 128, S)
            in_maps.append(
                dict(qT=qTs, kT=kTs, vT=vTs, WqT=WqT, WkT=WkT, WvT=WvT,
                     gq=gq, bq=bq, gk=gk, bk=bk)
            )
        res = run_bass_kernel_spmd(nc_a, in_maps, core_ids=list(range(NCORES)))
        for c in range(NCORES):
            xo = np.asarray(res.results[c]["xout"]).reshape(2, S, C)
            x_full[c, :, t0 : t0 + S, :] = xo

    out = np.empty((16, T, C), np.float32)
    xr_c = []
    for c in range(NCORES):
        x = x_full[c].reshape(2, T, H, DH)
        xr_c.append(x.transpose(0, 2, 1, 3).reshape(2 * T, C))
    for ms in range(2 * T // MS):
        m0 = ms * MS
        in_maps = []
        for c in range(NCORES):
            strip = xr_c[c][m0 : m0 + MS]
            xrTs = np.ascontiguousarray(strip.T).reshape(8, 128, MS)
            in_maps.append(dict(xrT=xrTs, WoT=WoT, bo=bof))
        res = run_bass_kernel_spmd(nc_b, in_maps, core_ids=list(range(NCORES)))
        for c in range(NCORES):
            oc = np.asarray(res.results[c]["o"])  # [MS, C]
            for half in range(2):
                lo = max(m0, half * T)
                hi = min(m0 + MS, (half + 1) * T)
                if lo < hi:
                    out[c + 8 * half, lo - half * T : hi - half * T] = oc[
                        lo - m0 : hi - m0
                    ]
    return out


def _kernel_numpy(q, k, v, Wq, Wk, Wv, Wo, bo, gamma, beta):
    B = q.shape[0]
    scale = 8.0 / DH
    reps = B // k.shape[0]
    k = np.tile(k, (reps, 1, 1))[:, :T]
    v = np.tile(v, (reps, 1, 1))[:, :T]
    out = np.empty((B, T, C), np.float32)
    for b in range(B):
        qp = (q[b] @ Wq.T).reshape(T, H, DH)
        kp = (k[b] @ Wk.T).reshape(T, H, DH)
        vp = (v[b] @ Wv.T).reshape(T, H, DH)

        def ln(x):
            mu = x.mean(-1, keepdims=True)
            var = ((x - mu) ** 2).mean(-1, keepdims=True)
            return (x - mu) / np.sqrt(var + EPS) * gamma + beta

        qp = ln(qp) * scale
        kp = ln(kp)
        attn = np.einsum("nhd,ngd->nhg", qp, kp)
        attn = attn - attn.max(-1, keepdims=True)
        attn = np.exp(attn)
        attn /= attn.sum(-1, keepdims=True)
        x = np.einsum("nhg,ngd->nhd", attn, vp)
        xr = x.transpose(1, 0, 2).reshape(T, C)
        out[b] = xr @ Wo.T + bo
    return out


def kernel(q, k, v, Wq, Wk, Wv, Wo, bo, gamma, beta):
    args = [np.asarray(a, np.float32) for a in (q, k, v, Wq, Wk, Wv, Wo, bo, gamma, beta)]
    try:
        return _kernel_device(*args)
    except Exception as e:
        import traceback

        traceback.print_exc()
        print("device path failed; using host fallback", flush=True)
        return _kernel_numpy(*args)



# revision 6
# speedup vs baseline: 2.8363x; 2.8363x over previous
import json
import numpy as np

import concourse.bass as bass
import concourse.mybir as mybir
import concourse.tile as tile
from concourse.bass_utils import run_bass_kernel_spmd

F32 = mybir.dt.float32
BF16 = mybir.dt.bfloat16
AX = mybir.AxisListType
AF = mybir.ActivationFunctionType
OP = mybir.AluOpType

H, DH, C, T = 16, 64, 1024, 2048
NT = T // 128          # k tiles per batch
NCORES = 8
EPS = 1e-5
SCALE = 8.0 / DH

# ---------------------------------------------------------------------------
# BIR fixup: this walrus build accepts at most ONE sync-wait per
# instruction; Tile's sem assignment attaches several. Split the excess
# onto NoOp carriers inserted just before, same engine/block (preserves
# per-engine program order => semantics).
# ---------------------------------------------------------------------------
_CTR = [0]


def _split_sync_waits(bir, max_waits=1):
    for fn in bir.get("functions", []):
        for blk in fn.get("blocks", []):
            insts = blk.get("instructions")
            if not insts:
                continue
            out = []
            changed = False
            for inst in insts:
                si = inst.get("sync_info")
                waits = si.get("on_wait") if si else None
                if waits and len(waits) > max_waits:
                    excess = waits[: len(waits) - max_waits]
                    si["on_wait"] = waits[len(waits) - max_waits:]
                    for i in range(0, len(excess), max_waits):
                        _CTR[0] += 1
                        out.append({
                            "debug": inst.get("debug", 0),
                            "engine": inst["engine"],
                            "ins": [], "outs": [],
                            "name": f"I-splitw-{_CTR[0]}",
                            "opcode": "NoOp",
                            "text_hint": "split_sync_wait",
                            "sync_info": {"on_update": [],
                                          "on_wait": excess[i:i + max_waits]},
                        })
                    changed = True
                out.append(inst)
            if changed:
                blk["instructions"] = out
    return bir


def _install_birfix():
    import concourse.bass2jax as b2j

    if getattr(b2j, "_birfix_installed", False):
        return
    orig = b2j._decompress_ant_bir

    def fixed(ant_bir_value):
        raw = orig(ant_bir_value)
        try:
            return json.dumps(_split_sync_waits(json.loads(raw))).encode()
        except Exception as e:  # fail open
            print(f"birfix failed ({e}); using original BIR", flush=True)
            return raw

    b2j._decompress_ant_bir = fixed
    b2j._birfix_installed = True


# ---------------------------------------------------------------------------
# AP helpers (broadcast axes on SBUF views)
# ---------------------------------------------------------------------------

def _ap(t, axes):
    """Build an AP on tile t with explicit [stride, num] free axes."""
    return bass.AP(tensor=t.tensor, offset=t.offset, ap=[t.ap[0], *axes])


# ---------------------------------------------------------------------------
# Program A: projections + LN + per-token HxH attention for 2 q batches
# sharing one k/v batch. Everything bf16; PSUM f32.
# ---------------------------------------------------------------------------

def build_prog_a():
    nc = bass.Bass(use_seq_codegen=True)
    qT = nc.dram_tensor("qT", [2 * NT, 128, C], BF16, kind="ExternalInput")
    kT = nc.dram_tensor("kT", [NT, 128, C], BF16, kind="ExternalInput")
    vT = nc.dram_tensor("vT", [NT, 128, C], BF16, kind="ExternalInput")
    wq = nc.dram_tensor("wq", [8, 128, C], BF16, kind="ExternalInput")
    wk = nc.dram_tensor("wk", [8, 128, C], BF16, kind="ExternalInput")
    wv = nc.dram_tensor("wv", [8, 128, C], BF16, kind="ExternalInput")
    gq = nc.dram_tensor("gq", [128, C], BF16, kind="ExternalInput")
    gk = nc.dram_tensor("gk", [128, C], BF16, kind="ExternalInput")
    xout = nc.dram_tensor("xout", [2 * T, C], BF16, kind="ExternalOutput")

    with tile.TileContext(nc) as tc:
        with (
            nc.allow_low_precision(reason="tolerance 2e-2; bf16 partials ok"),
            tc.tile_pool(name="wp", bufs=1) as wp,
            tc.tile_pool(name="act", bufs=3) as actp,
            tc.tile_pool(name="mid", bufs=2) as mid,
            tc.tile_pool(name="big", bufs=1) as big,
            tc.tile_pool(name="st", bufs=3) as st,
            tc.tile_pool(name="pp", bufs=2, space="PSUM") as pp,
        ):
            wsb = {}
            for nm, drt in (("q", wq), ("k", wk), ("v", wv)):
                w = wp.tile([128, 8, C], BF16, tag="w" + nm)
                for cb in range(8):
                    nc.sync.dma_start(out=w[:, cb, :], in_=drt[cb])
                wsb[nm] = w
            gq_sb = wp.tile([128, C], BF16, tag="gq")
            nc.sync.dma_start(out=gq_sb, in_=gq[:, :])
            gk_sb = wp.tile([128, C], BF16, tag="gk")
            nc.sync.dma_start(out=gk_sb, in_=gk[:, :])
            epst = wp.tile([128, 1], F32, tag="eps")
            nc.vector.memset(epst, EPS)

            # scratch for the two mega elementwise ops + fold tree
            P16 = big.tile([128, H * H * DH], BF16, tag="P16")      # 16384
            P8 = big.tile([128, H * H * DH // 2], BF16, tag="P8")   # 8192
            P4 = big.tile([128, H * H * DH // 4], BF16, tag="P4")   # 4096
            P2 = big.tile([128, H * H * DH // 8], BF16, tag="P2")   # 2048
            P1 = big.tile([128, H * H * DH // 16], BF16, tag="P1")  # 1024
            PH = big.tile([128, 512], BF16, tag="PH")               # 512

            def load_act(drt, idx, tag):
                a = actp.tile([128, C], BF16, tag=tag)
                nc.sync.dma_start(out=a, in_=drt[idx])
                return a

            def project(a, w):
                ps = pp.tile([128, C], F32, tag="pj")
                for cb in range(8):
                    for dh in range(2):
                        nc.tensor.matmul(
                            ps[:, dh * 512:(dh + 1) * 512],
                            lhsT=a[:, cb * 128:(cb + 1) * 128],
                            rhs=w[:, cb, dh * 512:(dh + 1) * 512],
                            start=(cb == 0),
                            stop=(cb == 7),
                        )
                return ps

            def ln_gamma(ps, g_sb, tag):
                """Centered projection ps -> (x*gamma_tile bf16, rstd bf16)."""
                xc = mid.tile([128, C], BF16, tag="xc" + tag)
                nc.scalar.copy(out=xc, in_=ps)
                sq = mid.tile([128, C], BF16, tag="sq" + tag)
                nc.scalar.activation(out=sq, in_=ps, func=AF.Square)
                vS = st.tile([128, H], F32, tag="vS" + tag)
                nc.vector.reduce_sum(
                    out=vS, in_=sq.rearrange("p (h d) -> p h d", d=DH), axis=AX.X
                )
                lnv = st.tile([128, H], F32, tag="lnv" + tag)
                nc.scalar.activation(
                    out=lnv, in_=vS, func=AF.Ln, bias=epst, scale=1.0 / DH
                )
                rstd = st.tile([128, H], BF16, tag="rstd" + tag)
                nc.scalar.activation(out=rstd, in_=lnv, func=AF.Exp, scale=-0.5)
                xg = mid.tile([128, C], BF16, tag="xg" + tag)
                nc.vector.tensor_tensor(out=xg, in0=xc, in1=g_sb, op=OP.mult)
                return xg, rstd

            def fold(src, dst, n_in):
                """dst[:, :, :n_in//2] = src[..., :half] + src[..., half:]"""
                half = n_in // 2
                s3 = src.rearrange("p (x d) -> p x d", d=n_in)
                d3 = dst.rearrange("p (x d) -> p x d", d=half)
                nc.vector.tensor_tensor(
                    out=d3, in0=s3[:, :, 0:half], in1=s3[:, :, half:n_in],
                    op=OP.add,
                )

            for kt in range(NT):
                ka = load_act(kT, kt, "ka")
                va = load_act(vT, kt, "va")
                kps = project(ka, wsb["k"])
                kg, rk = ln_gamma(kps, gk_sb, "k")
                vps = project(va, wsb["v"])
                vdm = mid.tile([128, C], BF16, tag="vdm")  # [p,(d,g)]
                nc.scalar.copy(out=vdm, in_=vps)
                for b in range(2):
                    qa = load_act(qT, b * NT + kt, "qa")
                    qps = project(qa, wsb["q"])
                    qg, rq = ln_gamma(qps, gq_sb, "q")

                    # QK: prod[p,(h,g,d)] = qg[p,h,d] * kg[p,g,d]
                    out3 = _ap(P16, [[H * DH, H], [DH, H], [1, DH]])
                    in0 = _ap(qg, [[DH, H], [0, H], [1, DH]])
                    in1 = _ap(kg, [[0, H], [DH, H], [1, DH]])
                    nc.vector.tensor_tensor(out=out3, in0=in0, in1=in1, op=OP.mult)
                    fold(P16, P8, DH)        # d: 64->32
                    fold(P8, P4, 32)         # 32->16
                    fold(P4, P2, 16)         # 16->8
                    fold(P2, P1, 8)          # 8->4
                    fold(P1, PH, 4)          # 4->2
                    s = st.tile([128, H * H], F32, tag="s")
                    sh3 = PH.rearrange("p (x d) -> p x d", d=2)
                    nc.vector.tensor_tensor(
                        out=s.rearrange("p (x d) -> p x d", d=1),
                        in0=sh3[:, :, 0:1], in1=sh3[:, :, 1:2], op=OP.add
                    )
                    s3 = s.rearrange("p (h g) -> p h g", g=H)
                    # logits *= rstd_q[h] * rstd_k[g]
                    nc.vector.tensor_tensor(
                        out=s3, in0=s3, in1=_ap(rq, [[1, H], [0, H]]), op=OP.mult
                    )
                    nc.vector.tensor_tensor(
                        out=s3, in0=s3, in1=_ap(rk, [[0, H], [1, H]]), op=OP.mult
                    )
                    # softmax over g (no max-sub; |logits| <= ~8)
                    eb = st.tile([128, H * H], BF16, tag="eb")
                    nc.scalar.activation(out=eb, in_=s, func=AF.Exp)
                    Z = st.tile([128, H], F32, tag="Z")
                    nc.vector.reduce_sum(
                        out=Z, in_=eb.rearrange("p (h g) -> p h g", g=H), axis=AX.X
                    )
                    zr = st.tile([128, H], BF16, tag="zr")
                    nc.vector.reciprocal(zr, Z)
                    at = st.tile([128, H * H], BF16, tag="at")
                    nc.vector.tensor_tensor(
                        out=at.rearrange("p (h g) -> p h g", g=H),
                        in0=eb.rearrange("p (h g) -> p h g", g=H),
                        in1=_ap(zr, [[1, H], [0, H]]),
                        op=OP.mult,
                    )
                    # AV: prod[p,(h,d,g)] = at[p,h,g] * vdm[p,d,g]
                    outv = _ap(P16, [[DH * H, H], [H, DH], [1, H]])
                    ia = _ap(at, [[H, H], [0, DH], [1, H]])
                    iv = _ap(vdm, [[0, H], [H, DH], [1, H]])
                    nc.vector.tensor_tensor(out=outv, in0=ia, in1=iv, op=OP.mult)
                    fold(P16, P8, H)         # g: 16->8
                    fold(P8, P4, 8)
                    fold(P4, P2, 4)
                    x = mid.tile([128, C], BF16, tag="x")
                    p23 = P2.rearrange("p (x d) -> p x d", d=2)
                    nc.vector.tensor_tensor(
                        out=x.rearrange("p (x d) -> p x d", d=1),
                        in0=p23[:, :, 0:1], in1=p23[:, :, 1:2], op=OP.add,
                    )
                    tok0 = b * T + kt * 128
                    nc.sync.dma_start(out=xout[tok0:tok0 + 128, :], in_=x)
    return nc


# ---------------------------------------------------------------------------
# Program B: output projection on the scrambled x rows.
# ---------------------------------------------------------------------------

def build_prog_b():
    nc = bass.Bass(use_seq_codegen=True)
    xs = nc.dram_tensor("xs", [2 * NT, 128, C], BF16, kind="ExternalInput")
    wo = nc.dram_tensor("wo", [8, 128, C], BF16, kind="ExternalInput")
    bo = nc.dram_tensor("bo", [128, C], F32, kind="ExternalInput")
    o = nc.dram_tensor("o", [2 * T, C], F32, kind="ExternalOutput")
    with tile.TileContext(nc) as tc:
        with (
            tc.tile_pool(name="wp", bufs=1) as wp,
            tc.tile_pool(name="act", bufs=3) as actp,
            tc.tile_pool(name="mid", bufs=3) as mid,
            tc.tile_pool(name="pp", bufs=2, space="PSUM") as pp,
        ):
            w = wp.tile([128, 8, C], BF16, tag="w")
            for cb in range(8):
                nc.sync.dma_start(out=w[:, cb, :], in_=wo[cb])
            bos = wp.tile([128, C], F32, tag="bo")
            nc.sync.dma_start(out=bos, in_=bo[:, :])
            for mt in range(2 * NT):
                a = actp.tile([128, C], BF16, tag="a")
                nc.sync.dma_start(out=a, in_=xs[mt])
                ps = pp.tile([128, C], F32, tag="pj")
                for cb in range(8):
                    for dh in range(2):
                        nc.tensor.matmul(
                            ps[:, dh * 512:(dh + 1) * 512],
                            lhsT=a[:, cb * 128:(cb + 1) * 128],
                            rhs=w[:, cb, dh * 512:(dh + 1) * 512],
                            start=(cb == 0),
                            stop=(cb == 7),
                        )
                osb = mid.tile([128, C], F32, tag="osb")
                nc.vector.scalar_tensor_tensor(
                    out=osb, in0=ps, scalar=1.0, in1=bos,
                    op0=OP.mult, op1=OP.add,
                )
                nc.sync.dma_start(out=o[mt * 128:(mt + 1) * 128, :], in_=osb)
    return nc


_PROGS = {}


def _get_progs():
    if "a" not in _PROGS:
        _install_birfix()
        _PROGS["a"] = build_prog_a()
        _PROGS["b"] = build_prog_b()
    return _PROGS["a"], _PROGS["b"]


def _tile_major(act):
    """[T?, C] -> [nt, 128, C] with partition-major lhsT layout.

    result[t, p, c*128+i] = act[t*128+i, c*128+p]
    """
    nt = act.shape[0] // 128
    r = act.reshape(nt, 128, 8, 128)          # [t, i, c, p]
    return np.ascontiguousarray(r.transpose(0, 3, 2, 1)).reshape(nt, 128, C)


def _center_w(W):
    """Per-head mean removal over d: makes projection output zero-mean."""
    Wr = W.reshape(H, DH, C)
    return (Wr - Wr.mean(axis=1, keepdims=True)).reshape(C, C)


def _kernel_device(q, k, v, Wq, Wk, Wv, Wo, bo, gamma, beta):
    if not np.all(beta == 0.0):
        raise RuntimeError("beta != 0 unsupported in device path")
    nc_a, nc_b = _get_progs()

    bf = lambda x: np.ascontiguousarray(x, dtype=mybir.dt.np(BF16))
    WqT = bf(_center_w(Wq).T.reshape(8, 128, C))
    WkT = bf(_center_w(Wk).T.reshape(8, 128, C))
    # v projection with (d, g)-major output channels:
    # vdm[t, d*16+g] = vp[t, g*64+d] -> permute Wv rows
    idx = (np.arange(C) % H) * DH + (np.arange(C) // H)   # row d*16+g <- g*64+d
    WvT = bf(Wv[idx].T.reshape(8, 128, C))
    WoT = bf(Wo.T.reshape(8, 128, C))
    gq_t = bf(np.broadcast_to(np.tile(gamma, H) * SCALE, (128, C)))
    gk_t = bf(np.broadcast_to(np.tile(gamma, H), (128, C)))
    bo_t = np.ascontiguousarray(np.broadcast_to(bo, (128, C)), np.float32)

    in_a = []
    for c in range(NCORES):
        qT = np.concatenate(
            [_tile_major(bf(q[c])), _tile_major(bf(q[c + 8]))], axis=0
        )
        kT = _tile_major(bf(k[c % 4]))
        vT = _tile_major(bf(v[c % 4]))
        in_a.append(dict(qT=qT, kT=kT, vT=vT, wq=WqT, wk=WkT, wv=WvT,
                         gq=gq_t, gk=gk_t))
    res_a = run_bass_kernel_spmd(nc_a, in_a, core_ids=list(range(NCORES)))

    # host scramble: y[128h+u, 64j+d] = x[16u+j, h, d]
    in_b = []
    for c in range(NCORES):
        xo = np.asarray(res_a.results[c]["xout"]).reshape(2, T, H, DH)
        ys = []
        for half in range(2):
            x4 = xo[half].reshape(128, 16, H, DH)          # [u, j, h, d]
            y = np.ascontiguousarray(x4.transpose(2, 0, 1, 3)).reshape(T, C)
            ys.append(y)
        xs = np.concatenate([_tile_major(y) for y in ys], axis=0)
        in_b.append(dict(xs=xs, wo=WoT, bo=bo_t))
    res_b = run_bass_kernel_spmd(nc_b, in_b, core_ids=list(range(NCORES)))

    out = np.empty((16, T, C), np.float32)
    for c in range(NCORES):
        oc = np.asarray(res_b.results[c]["o"])
        out[c] = oc[:T]
        out[c + 8] = oc[T:]
    return out


def _kernel_numpy(q, k, v, Wq, Wk, Wv, Wo, bo, gamma, beta):
    B = q.shape[0]
    reps = B // k.shape[0]
    k = np.tile(k, (reps, 1, 1))[:, :T]
    v = np.tile(v, (reps, 1, 1))[:, :T]
    out = np.empty((B, T, C), np.float32)
    for b in range(B):
        qp = (q[b] @ Wq.T).reshape(T, H, DH)
        kp = (k[b] @ Wk.T).reshape(T, H, DH)
        vp = (v[b] @ Wv.T).reshape(T, H, DH)

        def ln(x):
            mu = x.mean(-1, keepdims=True)
            var = ((x - mu) ** 2).mean(-1, keepdims=True)
            return (x - mu) / np.sqrt(var + EPS) * gamma + beta

        qp = ln(qp) * SCALE
        kp = ln(kp)
        attn = np.einsum("nhd,ngd->nhg", qp, kp)
        attn = attn - attn.max(-1, keepdims=True)
        attn = np.exp(attn)
        attn /= attn.sum(-1, keepdims=True)
        x = np.einsum("nhg,ngd->nhd", attn, vp)
        xr = x.transpose(1, 0, 2).reshape(T, C)
        out[b] = xr @ Wo.T + bo
    return out


def kernel(q, k, v, Wq, Wk, Wv, Wo, bo, gamma, beta):
    args = [np.asarray(a, np.float32)
            for a in (q, k, v, Wq, Wk, Wv, Wo, bo, gamma, beta)]
    try:
        return _kernel_device(*args)
    except Exception:
        import traceback

        traceback.print_exc()
        print("device path failed; using host fallback", flush=True)
        return _kernel_numpy(*args)
